# revision 12
# baseline (speedup 1.0000x reference)
"""Bass/Trainium2 kernel for nn_Head_13030930776875.

out = 0.7*softmax(causal(x@Wq @ (x@Wk)^T / sqrt(d))) @ (x@Wv)
    + 0.3*rownorm(causal(exp(-|y_i - y_j|^2 / (2d)))) @ (x@Wv),  y = (x@Wk)@L_grav

Sharding: 8 cores = 4 samples x 2 halves. Each half owns two 512-row query
groups chosen so causal (triangular) work balances: half0 -> {G0, G3},
half1 -> {G1, G2}. The program is SPMD-uniform; per-half differences are
carried in input data (masks, exp-bias gates of -1e30).

On-device layout: everything transposed (d on partitions). Scores are
computed as s^T tiles [k,q] so that (a) A^T slices feed the A@v matmul
directly as the stationary operand (no transposes anywhere), and (b) the
causal row-sums come free via a ones-column appended to v. The grav kernel
exp(-dist2/2d) is factorized; the per-query factor cancels in the row
normalization and the per-key factor -sq_k/256 rides the ACT exp bias.
"""

import math
import os

import numpy as np

B, N, D_MODEL, D_HEAD = 4, 2048, 1024, 128
OMEGA_LANG, OMEGA_GRAV = 0.7, 0.3
SC_LANG = 1.0 / math.sqrt(D_HEAD)
SC_GRAV = 1.0 / D_HEAD
NEG = -1.0e30
NBLK = N // 128            # 16 k-chunks of 128
NCH = (8, 16)              # chunks per position (pos0 group, pos1 group)
NSLOT = NCH[0] + NCH[1]    # 24 mask/bias slots

_CACHE = {}


def _build_nc():
    import concourse.bacc as bacc
    import concourse.mybir as mybir
    import concourse.tile as tile
    import concourse.bass as bass

    dt = mybir.dt
    F16, F32 = dt.float16, dt.float32
    AF = mybir.ActivationFunctionType
    OP = mybir.AluOpType

    nc = bacc.Bacc()

    # Packed inputs: each is one contiguous-per-partition DMA (128 fat
    # descriptors) instead of many 1KB-descriptor transfers. Issue order on
    # the sync HWDGE queue = program order below: smalls first so the PE
    # warmup can start early, then weights, then x in group-major order
    # interleaved with the query-column copies.
    sm16 = nc.declare_dram_parameter("sm16", [128, 640], F16, isOutput=False)
    sm32 = nc.declare_dram_parameter("sm32", [128, 3 * NSLOT], F32, isOutput=False)
    wpack = nc.declare_dram_parameter("wpack", [128, 4 * 1024], F16, isOutput=False)
    # xg[g*128+p, c*512+n] = x[b].T[c*128+p, g*512+n], one row-block per group
    xg = nc.declare_dram_parameter("xg", [4 * 128, 8 * 512], F16, isOutput=False)
    xqg = nc.declare_dram_parameter("xqg", [2 * 128, 8 * 512], F16, isOutput=False)
    out_d = nc.declare_dram_parameter("out", [N // 2, 128], F32, isOutput=True)

    with tile.TileContext(nc) as tc:
        with (
            tc.tile_pool(name="big", bufs=1) as big,
            tc.tile_pool(name="xtp", bufs=1) as xtp,
            tc.tile_pool(name="xqp", bufs=1) as xqp,
            tc.tile_pool(name="ap", bufs=2) as apool,
            tc.tile_pool(name="small", bufs=4) as small,
            tc.tile_pool(name="outp", bufs=4) as outp,
            tc.tile_pool(name="score", bufs=4, space="PSUM") as score,
            tc.tile_pool(name="pp", bufs=4, space="PSUM") as pp,
        ):
            # ---- small inputs first (gate the PE warmup + masks) ----
            sm16_s = big.tile([128, 640], F16, tag="sm16")
            nc.sync.dma_start(sm16_s[:], sm16[:])
            lg_s = sm16_s[:, 0:128]
            io_s = sm16_s[:, 128:640]
            sm32_s = big.tile([128, 3 * NSLOT], F32, tag="sm32")
            nc.sync.dma_start(sm32_s[:], sm32[:])
            th_s = sm32_s[:, 0:NSLOT]
            lb_s = sm32_s[:, NSLOT:2 * NSLOT]
            gb_s = sm32_s[:, 2 * NSLOT:3 * NSLOT]
            wp_s = big.tile([128, 4, 8, 128], F16, tag="wp")
            nc.sync.dma_start(wp_s[:], wpack[:].rearrange("p (w c d) -> p w c d",
                                                          w=4, c=8))

            # ---- x^T group-major: one DMA per 512-col group (g, c, n) ----
            xt_all = xtp.tile([128, 4, 8, 512], F16, tag="xt")
            xq_all = xqp.tile([128, 2, 8, 512], F16, tag="xq")
            for g in range(4):
                nc.sync.dma_start(
                    xt_all[:, g],
                    xg[g * 128:(g + 1) * 128, :].rearrange("p (c n) -> p c n", c=8))
                if g < 2:
                    nc.sync.dma_start(
                        xq_all[:, g],
                        xqg[g * 128:(g + 1) * 128, :].rearrange("p (c n) -> p c n",
                                                                c=8))

            # ---- projections, emitted group-sliced so pos0 unblocks early ----
            kT = big.tile([128, N], F16, tag="kT")
            qT = big.tile([128, N // 2], F16, tag="qT")
            yqT = big.tile([128, N // 2], F16, tag="yqT")
            yT = big.tile([128, N], F16, tag="yT")
            sqn = big.tile([128, NBLK], F32, tag="sqn")
            gvb = big.tile([128, NSLOT], F32, tag="gvb")
            vaug = big.tile([128, NBLK, 132], F16, tag="vaug")

            def proj_group(dst, wi, src_all, g):
                cols = slice(g * 512, (g + 1) * 512)
                ps = pp.tile([128, 512], F32, tag="pp")
                for c in range(8):
                    nc.tensor.matmul(ps[:], wp_s[:, wi, c, :], src_all[:, g, c, :],
                                     start=(c == 0), stop=(c == 7))
                nc.vector.tensor_copy(dst[:, cols], ps[:])

            def yt_group(g):
                cols = slice(g * 512, (g + 1) * 512)
                ps = pp.tile([128, 512], F32, tag="pp")
                nc.tensor.matmul(ps[:], lg_s, kT[:, cols])
                nc.vector.tensor_copy(yT[:, cols], ps[:])

            def sqn_chunk(kb):
                ps = pp.tile([128, 512], F32, tag="pp")
                nc.tensor.matmul(ps[:, 0:128], kT[:, kb * 128:(kb + 1) * 128], lg_s)
                scr = small.tile([128, 128], F32, tag="scr")
                nc.scalar.activation(scr[:], ps[:, 0:128], AF.Square,
                                     scale=0.0625, accum_out=sqn[:, kb:kb + 1])

            def vaug_chunk(kb):
                g, blk = kb // 4, kb % 4
                ps = pp.tile([128, 512], F32, tag="pp")
                for c in range(8):
                    nc.tensor.matmul(ps[:, 0:128],
                                     xt_all[:, g, c, blk * 128:(blk + 1) * 128],
                                     wp_s[:, 2, c, :], start=(c == 0), stop=(c == 7))
                nc.vector.tensor_copy(vaug[:, kb, 0:128], ps[:, 0:128])
                nc.vector.memset(vaug[:, kb, 128:129], 1.0)

            # PE warmup: ~3.5us of dummy matmuls on lg so the HAM clock-gate
            # opens (4/8 -> 8/8) before the real stream begins.
            warm = pp.tile([128, 512], F32, tag="pp")
            for i in range(28):
                nc.tensor.matmul(warm[:, 0:128], lg_s, lg_s,
                                 start=(i == 0), stop=(i == 27))

            # device-generated causal masks: mk[slot] = (iota >= thr[slot])
            mk_s = big.tile([128, NSLOT * 512], F16, tag="mk")
            for slot in range(NSLOT):
                nc.vector.tensor_scalar(mk_s[:, slot * 512:(slot + 1) * 512],
                                        io_s, th_s[:, slot:slot + 1], None,
                                        OP.is_ge)

            # pos0 prerequisites: kT g0-g1, qT/yqT g0, yT g0-g1, sqn/gvb 0..7
            proj_group(kT, 1, xt_all, 0)
            proj_group(kT, 1, xt_all, 1)
            proj_group(qT, 0, xq_all, 0)
            yt_group(0)
            yt_group(1)
            proj_group(yqT, 3, xq_all, 0)
            for kb in range(8):
                sqn_chunk(kb)
            nc.vector.tensor_tensor(gvb[:, 0:8], gb_s[:, 0:8], sqn[:, 0:8], OP.subtract)
            for kb in range(8):
                vaug_chunk(kb)

            # ---- attention, per position (two query groups of 512) ----
            for pos in range(2):
                if pos == 1:
                    proj_group(kT, 1, xt_all, 2)
                    proj_group(kT, 1, xt_all, 3)
                    proj_group(qT, 0, xq_all, 1)
                    yt_group(2)
                    yt_group(3)
                    proj_group(yqT, 3, xq_all, 1)
                    for kb in range(8, 16):
                        sqn_chunk(kb)
                    nc.vector.tensor_tensor(gvb[:, 8:24], gb_s[:, 8:24],
                                            sqn[:, 0:16], OP.subtract)
                    for kb in range(8, 16):
                        vaug_chunk(kb)
                nch = NCH[pos]
                slot0 = 0 if pos == 0 else NCH[0]
                qoff = pos * 512
                alang = apool.tile([128, NCH[1] * 512], F16, tag="alang")
                agrav = apool.tile([128, NCH[1] * 512], F16, tag="agrav")
                for kb in range(nch):
                    if pos == 0:
                        shrink = 0 if kb < 4 else (kb - 4) * 128
                    else:
                        shrink = 0 if kb < 12 else (kb - 12) * 128
                    w = 512 - shrink
                    slot = slot0 + kb
                    kcols = slice(kb * 128, (kb + 1) * 128)
                    acols = slice(kb * 512 + shrink, (kb + 1) * 512)
                    # lang: s^T = k_blk @ q^T
                    ps = score.tile([128, 512], F32, tag="sc")
                    nc.tensor.matmul(ps[:, 0:w], kT[:, kcols],
                                     qT[:, qoff + shrink:qoff + 512])
                    nc.scalar.activation(alang[:, acols], ps[:, 0:w], AF.Exp,
                                         bias=lb_s[:, slot:slot + 1], scale=SC_LANG)
                    # grav: gram^T = y_blk @ yq^T ; kern~ = exp(gram/128 - sq_k/256)
                    pg = score.tile([128, 512], F32, tag="sc")
                    nc.tensor.matmul(pg[:, 0:w], yT[:, kcols],
                                     yqT[:, qoff + shrink:qoff + 512])
                    nc.scalar.activation(agrav[:, acols], pg[:, 0:w], AF.Exp,
                                         bias=gvb[:, slot:slot + 1], scale=SC_GRAV)
                    # causal mask multiply (2x-mode TT against device-built mask)
                    if pos == 0 or kb >= 8:
                        mcols = slice(slot * 512 + shrink, (slot + 1) * 512)
                        nc.vector.tensor_tensor(alang[:, acols], alang[:, acols],
                                                mk_s[:, mcols], OP.mult)
                        nc.vector.tensor_tensor(agrav[:, acols], agrav[:, acols],
                                                mk_s[:, mcols], OP.mult)
                # A^T @ v_aug per 128-row query block
                obp = outp.tile([128, 4, 128], F32, tag="obp")
                for j in range(4):
                    nkb = (5 + j) if pos == 0 else (13 + j)
                    pol = pp.tile([128, 132], F32, tag="pp")
                    pog = pp.tile([128, 132], F32, tag="pp")
                    for kb in range(nkb):
                        nc.tensor.matmul(pol[:, 0:129],
                                         alang[:, kb * 512 + j * 128:kb * 512 + (j + 1) * 128],
                                         vaug[:, kb, 0:129],
                                         start=(kb == 0), stop=(kb == nkb - 1))
                    for kb in range(nkb):
                        nc.tensor.matmul(pog[:, 0:129],
                                         agrav[:, kb * 512 + j * 128:kb * 512 + (j + 1) * 128],
                                         vaug[:, kb, 0:129],
                                         start=(kb == 0), stop=(kb == nkb - 1))
                    rl = small.tile([128, 1], F32, tag="rl")
                    rg = small.tile([128, 1], F32, tag="rg")
                    nc.vector.reciprocal(rl[:], pol[:, 128:129])
                    nc.vector.tensor_scalar(rl[:], rl[:], OMEGA_LANG, None, OP.mult)
                    nc.vector.reciprocal(rg[:], pog[:, 128:129])
                    nc.vector.tensor_scalar(rg[:], rg[:], OMEGA_GRAV, None, OP.mult)
                    ob = outp.tile([128, 128], F32, tag="ob")
                    nc.vector.tensor_scalar(ob[:], pol[:, 0:128], rl[:], None, OP.mult)
                    nc.vector.scalar_tensor_tensor(obp[:, j, :], pog[:, 0:128], rg[:],
                                                   ob[:], OP.mult, OP.add)
                r0 = pos * 512
                nc.sync.dma_start(
                    out_d[r0:r0 + 512, :].rearrange("(j p) d -> p j d", j=4),
                    obp[:])

    nc.finalize()
    return nc


def _host_inputs(x, Wq, Wk, Wv, L_grav):
    """Build the 8 per-core input maps."""
    f16 = np.float16
    x = np.asarray(x, np.float32)
    Wq = np.asarray(Wq, np.float32)
    Wk = np.asarray(Wk, np.float32)
    Wv = np.asarray(Wv, np.float32)
    L = np.asarray(L_grav, np.float32)
    Wy = Wk @ L

    def warr(w):  # [1024,128] -> [128, 8*128] chunk-major for lhsT slices
        return np.ascontiguousarray(
            w.reshape(8, 128, 128).transpose(1, 0, 2).reshape(128, 8 * 128)
        ).astype(f16)

    # wpack [128, 4*1024]: wq|wk|wv|wy, each chunk-major [128, 8*128]
    wpack = np.concatenate([warr(Wq), warr(Wk), warr(Wv), warr(Wy)],
                           axis=1)
    lga = L.astype(f16)

    iota = np.ascontiguousarray(
        np.broadcast_to(np.arange(512, dtype=np.float32), (128, 512))).astype(f16)
    # sm16 [128, 640] = lg | iota
    sm16 = np.concatenate([lga, iota], axis=1)

    def pack_groups(xTb, ngroups):
        """[1024, ng*512] -> [ng*128, 8*512]: row g*128+p, col c*512+n."""
        t = xTb.reshape(8, 128, ngroups, 512)          # (c, p, g, n)
        return np.ascontiguousarray(
            t.transpose(2, 1, 0, 3).reshape(ngroups * 128, 8 * 512))

    def half_data(h):
        """thr [128,24] (mask = iota >= thr), lbias/gbias [128,24] for half h."""
        p = np.arange(128, dtype=np.float32)
        th = np.empty((128, 24), np.float32)
        lb = np.zeros(24, np.float32)
        gb = np.zeros(24, np.float32)
        for pos in range(2):
            nch = NCH[pos]
            slot0 = 0 if pos == 0 else NCH[0]
            gs = ((0, 12) if h == 0 else (4, 8))[pos]
            for kb in range(nch):
                slot = slot0 + kb
                m = kb - gs  # chunk index relative to group start
                if m < 0:
                    th[:, slot] = -1e9  # fully valid
                elif m < 4:
                    th[:, slot] = m * 128 + p  # causal band (group-local cols)
                else:
                    th[:, slot] = 1e9  # fully invalid (also bias-gated)
                    lb[slot] = NEG
                    gb[slot] = NEG
        lbf = np.broadcast_to(lb, (128, 24)).astype(np.float32).copy()
        gbf = np.broadcast_to(gb, (128, 24)).astype(np.float32).copy()
        return th, lbf, gbf

    halves = [half_data(0), half_data(1)]
    in_maps = []
    for core in range(8):
        b, h = core // 2, core % 2
        xTb = np.ascontiguousarray(x[b].T).astype(f16)  # [1024, 2048]
        if h == 0:
            xq = np.concatenate([xTb[:, 0:512], xTb[:, 1536:2048]], axis=1)
        else:
            xq = np.ascontiguousarray(xTb[:, 512:1536])
        th, lbf, gbf = halves[h]
        sm32 = np.concatenate([th, lbf, gbf], axis=1)
        in_maps.append({
            "sm16": sm16, "sm32": sm32, "wpack": wpack,
            "xg": pack_groups(xTb, 4), "xqg": pack_groups(xq, 2),
        })
    return in_maps


def kernel(x, Wq, Wk, Wv, L_grav):
    import concourse.bass_utils as bass_utils

    if "nc" not in _CACHE:
        _CACHE["nc"] = _build_nc()
    nc = _CACHE["nc"]
    in_maps = _host_inputs(x, Wq, Wk, Wv, L_grav)

    trace = bool(os.environ.get("BASS_KERNEL_TRACE"))
    if trace:
        bass_utils.upload_artifacts = lambda tmpdir: f"file://{tmpdir}"
    res = bass_utils.run_bass_kernel_spmd(nc, in_maps, list(range(8)), trace=trace)
    if trace:
        _CACHE["exec_time_ns"] = res.exec_time_ns
        _CACHE["mean_exec_time_ns"] = res.mean_exec_time_ns

    out = np.empty((B, N, D_HEAD), np.float32)
    for core in range(8):
        b, h = core // 2, core % 2
        r = res.results[core]["out"]
        if h == 0:
            out[b, 0:512] = r[0:512]
            out[b, 1536:2048] = r[512:1024]
        else:
            out[b, 512:1024] = r[0:512]
            out[b, 1024:1536] = r[512:1024]
    return out



# revision 13
# speedup vs baseline: 1.1416x; 1.1416x over previous
"""Bass/Trainium2 kernel for nn_Head_13030930776875 (v3: interleaved).

out = 0.7*softmax(causal(q k^T / sqrt(d))) @ v
    + 0.3*rownorm(causal(exp(-|y_i-y_j|^2 / (2d)))) @ v,   y = k @ L_grav

Sharding: 8 cores = 4 samples x 2 halves. Half h owns query blocks
{h, h+2, ..., h+14} (128-row blocks, stride-2 interleave) — rank-matched
causal needs, so the uniform trim covers both halves with only 72 of the
136 possible 128x128 score tiles per attention (optimal for 2 cores).

Per-core key order is PERMUTED host-side (group g packed as
[4g+h, 4g+1-h, 4g+2+h, 4g+3-h]) so the owned query blocks sit at even
packed positions — queries are strided views of x^T / y^T, no separate
query tensors. Attention over keys is order-invariant; causal masks are
per-core data (one [128,128] mask slot per chunk covers the single
band-or-invalid leftmost sub-block).

Scales are folded host-side (Wq/sqrt(d), L/sqrt(d)) so lang+grav exp
merge into ONE bias-free ACT call per chunk over adjacent PSUM banks;
the grav per-key factor exp(-sq_k/2) rides a g-scaled copy of v_aug.
"""

import math
import os

import numpy as np

B, N, D_MODEL, D_HEAD = 4, 2048, 1024, 128
OMEGA_LANG, OMEGA_GRAV = 0.7, 0.3
NBLK = 16

_CACHE = {}


def _build_nc():
    import concourse.bacc as bacc
    import concourse.mybir as mybir
    import concourse.tile as tile

    dt = mybir.dt
    F16, F32 = dt.float16, dt.float32
    AF = mybir.ActivationFunctionType
    OP = mybir.AluOpType

    nc = bacc.Bacc()

    sm16 = nc.declare_dram_parameter("sm16", [128, 256], F16, isOutput=False)
    sm32 = nc.declare_dram_parameter("sm32", [128, 16], F32, isOutput=False)
    wpack = nc.declare_dram_parameter("wpack", [128, 3 * 1024], F16, isOutput=False)
    # xg[g*128+p, ((c*4+s)*128)+n] = xT[c*128+p, P_g[s]*128+n], permuted blocks
    xg = nc.declare_dram_parameter("xg", [4 * 128, 8 * 512], F16, isOutput=False)
    out_d = nc.declare_dram_parameter("out", [N // 2, 128], F32, isOutput=True)

    with tile.TileContext(nc) as tc:
        with (
            tc.tile_pool(name="big", bufs=1) as big,
            tc.tile_pool(name="xtp", bufs=1) as xtp,
            tc.tile_pool(name="ap", bufs=2) as apool,
            tc.tile_pool(name="small", bufs=4) as small,
            tc.tile_pool(name="outp", bufs=2) as outp,
            tc.tile_pool(name="score", bufs=2, space="PSUM") as score,
            tc.tile_pool(name="pp", bufs=4, space="PSUM") as pp,
        ):
            # ---- small inputs first (gate PE warmup + masks) ----
            sm16_s = big.tile([128, 256], F16, tag="sm16")
            nc.sync.dma_start(sm16_s[:], sm16[:])
            lg_s = sm16_s[:, 0:128]
            io_s = sm16_s[:, 128:256]
            th_s = big.tile([128, 16], F32, tag="th")
            nc.sync.dma_start(th_s[:], sm32[:])
            wp_s = big.tile([128, 3, 8, 128], F16, tag="wp")
            nc.sync.dma_start(wp_s[:], wpack[:].rearrange("p (w c d) -> p w c d",
                                                          w=3, c=8))
            # x^T permuted-block layout (g, c, j, o, n): o=0 are owned q blocks
            xt_all = xtp.tile([128, 4, 8, 2, 2, 128], F16, tag="xt")
            for g in range(4):
                nc.sync.dma_start(
                    xt_all[:, g],
                    xg[g * 128:(g + 1) * 128, :].rearrange(
                        "p (c j o n) -> p c j o n", c=8, j=2, o=2))

            kT = big.tile([128, 16, 128], F16, tag="kT")
            yT = big.tile([128, 8, 2, 128], F16, tag="yT")
            qT = big.tile([128, 1024], F16, tag="qT")
            sqn = big.tile([128, NBLK], F32, tag="sqn")
            g_s = big.tile([128, NBLK], F32, tag="gs")
            vaug = big.tile([128, NBLK, 132], F16, tag="vaug")
            vaug_g = big.tile([128, NBLK, 132], F16, tag="vaugg")

            # PE warmup (~2.5us of dummy matmuls so the HAM clock-gate opens)
            warm = pp.tile([128, 512], F32, tag="pp")
            for i in range(24):
                nc.tensor.matmul(warm[:, 0:128], lg_s, lg_s,
                                 start=(i == 0), stop=(i == 23))

            # causal masks: mk[slot] = (iota128 >= thr[slot]), 16 slots
            mk_s = big.tile([128, 16, 128], F16, tag="mk")
            for slot in range(16):
                nc.vector.tensor_scalar(mk_s[:, slot, :], io_s,
                                        th_s[:, slot:slot + 1], None, OP.is_ge)

            def proj_kt(g):
                ps = pp.tile([128, 512], F32, tag="pp")
                for c in range(8):
                    nc.tensor.matmul(ps[:], wp_s[:, 1, c, :], xt_all[:, g, c],
                                     start=(c == 0), stop=(c == 7))
                nc.vector.tensor_copy(kT[:, 4 * g:4 * (g + 1), :], ps[:])

            def proj_qt(half):
                ps = pp.tile([128, 512], F32, tag="pp")
                for c in range(8):
                    nc.tensor.matmul(ps[:],
                                     wp_s[:, 0, c, :],
                                     xt_all[:, 2 * half:2 * half + 2, c, :, 0, :],
                                     start=(c == 0), stop=(c == 7))
                nc.vector.tensor_copy(qT[:, half * 512:(half + 1) * 512], ps[:])

            def yt_group(g):
                ps = pp.tile([128, 512], F32, tag="pp")
                nc.tensor.matmul(ps[:], lg_s, kT[:, 4 * g:4 * (g + 1), :])
                nc.vector.tensor_copy(yT[:, 2 * g:2 * (g + 1), :, :], ps[:])

            def sqn_chunk(kb):
                ps = pp.tile([128, 512], F32, tag="pp")
                nc.tensor.matmul(ps[:, 0:128], kT[:, kb, :], lg_s)
                scr = small.tile([128, 128], F32, tag="scr")
                nc.scalar.activation(scr[:], ps[:, 0:128], AF.Square,
                                     scale=0.70710678, accum_out=sqn[:, kb:kb + 1])

            def vaug_chunk(kb):
                g, s = kb // 4, kb % 4
                ps = pp.tile([128, 512], F32, tag="pp")
                for c in range(8):
                    nc.tensor.matmul(ps[:, 0:128],
                                     xt_all[:, g, c, s // 2, s % 2, :],
                                     wp_s[:, 2, c, :], start=(c == 0), stop=(c == 7))
                nc.vector.tensor_copy(vaug[:, kb, 0:128], ps[:, 0:128])
                nc.vector.memset(vaug[:, kb, 128:129], 1.0)
                nc.vector.tensor_scalar(vaug_g[:, kb, 0:129], vaug[:, kb, 0:129],
                                        g_s[:, kb:kb + 1], None, OP.mult)

            # pos0 prerequisites
            proj_kt(0)
            proj_kt(1)
            proj_qt(0)
            yt_group(0)
            yt_group(1)
            for kb in range(8):
                sqn_chunk(kb)
            nc.scalar.activation(g_s[:, 0:8], sqn[:, 0:8], AF.Exp, scale=-1.0)
            for kb in range(8):
                vaug_chunk(kb)

            # ---- attention: pos0 = local query blocks 0..3, pos1 = 4..7 ----
            for pos in range(2):
                if pos == 1:
                    proj_kt(2)
                    proj_kt(3)
                    proj_qt(1)
                    yt_group(2)
                    yt_group(3)
                    for kb in range(8, 16):
                        sqn_chunk(kb)
                    nc.scalar.activation(g_s[:, 8:16], sqn[:, 8:16], AF.Exp,
                                         scale=-1.0)
                    for kb in range(8, 16):
                        vaug_chunk(kb)
                nch = 8 if pos == 0 else 16
                amrg = apool.tile([128, 16, 1024], F16, tag="amrg")
                for kb in range(nch):
                    if pos == 0:
                        shrink = (kb // 2) * 128
                    else:
                        shrink = max(0, kb // 2 - 4) * 128
                    qsl = qT[:, pos * 512 + shrink:pos * 512 + 512]
                    i0 = pos * 4 + shrink // 128
                    ysl = yT[:, i0:pos * 4 + 4, 0, :]
                    sc = score.tile([128, 1024], F32, tag="sc")
                    nc.tensor.matmul(sc[:, shrink:512], kT[:, kb, :], qsl)
                    nc.tensor.matmul(sc[:, 512 + shrink:1024],
                                     yT[:, kb // 2, kb % 2, :], ysl)
                    nc.scalar.activation(amrg[:, kb, shrink:1024],
                                         sc[:, shrink:1024], AF.Exp)
                    # leftmost included sub-block is band/invalid/full per-core
                    if pos == 0 or kb >= 8:
                        slot = kb
                        nc.vector.tensor_tensor(
                            amrg[:, kb, shrink:shrink + 128],
                            amrg[:, kb, shrink:shrink + 128],
                            mk_s[:, slot, :], OP.mult)
                        nc.vector.tensor_tensor(
                            amrg[:, kb, 512 + shrink:512 + shrink + 128],
                            amrg[:, kb, 512 + shrink:512 + shrink + 128],
                            mk_s[:, slot, :], OP.mult)
                # A^T @ v_aug per owned 128-row query block
                obp = outp.tile([128, 4, 128], F32, tag="obp")
                for jj in range(4):
                    nkb = 2 * jj + 2 if pos == 0 else 2 * jj + 10
                    pol = pp.tile([128, 132], F32, tag="pp")
                    pog = pp.tile([128, 132], F32, tag="pp")
                    for kb in range(nkb):
                        nc.tensor.matmul(pol[:, 0:129],
                                         amrg[:, kb, jj * 128:(jj + 1) * 128],
                                         vaug[:, kb, 0:129],
                                         start=(kb == 0), stop=(kb == nkb - 1))
                    for kb in range(nkb):
                        nc.tensor.matmul(pog[:, 0:129],
                                         amrg[:, kb, 512 + jj * 128:512 + (jj + 1) * 128],
                                         vaug_g[:, kb, 0:129],
                                         start=(kb == 0), stop=(kb == nkb - 1))
                    rl = small.tile([128, 1], F32, tag="rl")
                    rg = small.tile([128, 1], F32, tag="rg")
                    nc.vector.reciprocal(rl[:], pol[:, 128:129])
                    nc.vector.tensor_scalar(rl[:], rl[:], OMEGA_LANG, None, OP.mult)
                    nc.vector.reciprocal(rg[:], pog[:, 128:129])
                    nc.vector.tensor_scalar(rg[:], rg[:], OMEGA_GRAV, None, OP.mult)
                    ob = outp.tile([128, 128], F32, tag="ob")
                    nc.vector.tensor_scalar(ob[:], pol[:, 0:128], rl[:], None,
                                            OP.mult)
                    nc.vector.scalar_tensor_tensor(obp[:, jj, :], pog[:, 0:128],
                                                   rg[:], ob[:], OP.mult, OP.add)
                nc.sync.dma_start(
                    out_d[pos * 512:(pos + 1) * 512, :].rearrange(
                        "(j p) d -> p j d", j=4),
                    obp[:])

    nc.finalize()
    return nc


def _host_inputs(x, Wq, Wk, Wv, L_grav):
    """Build the 8 per-core input maps."""
    f16 = np.float16
    x = np.asarray(x, np.float32)
    s = 1.0 / math.sqrt(D_HEAD)
    Wq = np.asarray(Wq, np.float32) * s        # fold 1/sqrt(d) into Wq
    Wk = np.asarray(Wk, np.float32)
    Wv = np.asarray(Wv, np.float32)
    L = np.asarray(L_grav, np.float32) * s     # fold 1/sqrt(d) into L

    def warr(w):  # [1024,128] -> [128, 8*128] chunk-major for lhsT slices
        return np.ascontiguousarray(
            w.reshape(8, 128, 128).transpose(1, 0, 2).reshape(128, 8 * 128)
        ).astype(f16)

    wpack = np.concatenate([warr(Wq), warr(Wk), warr(Wv)], axis=1)
    iota = np.ascontiguousarray(
        np.broadcast_to(np.arange(128, dtype=np.float32), (128, 128))).astype(f16)
    sm16 = np.concatenate([L.astype(f16), iota], axis=1)

    def perm(h):  # packed block order within each group
        return [h, 1 - h, 2 + h, 3 - h]

    def half_thr(h):
        """thr [128, 16]: mask = iota128 >= thr, one slot per masked chunk.

        Slot kb<8 -> pos0 chunk kb; slot 8+s -> pos1 chunk 8+s. The slot
        masks the leftmost included sub-block: local i0 = chunk//2, global
        query block G = 4*(i0//2) + h + 2*(i0%2); key block K from perm."""
        p = np.arange(128, dtype=np.float32)
        th = np.empty((128, 16), np.float32)
        pm = perm(h)
        for slot in range(16):
            kb = slot  # slots 0..7: pos0 chunks; slots 8..15: pos1 chunks
            i0 = kb // 2
            G = 4 * (i0 // 2) + h + 2 * (i0 % 2)
            K = 4 * (kb // 4) + pm[kb % 4]
            if G > K:
                th[:, slot] = -1e9          # fully valid
            elif G == K:
                th[:, slot] = p             # causal band diagonal
            else:
                th[:, slot] = 1e9           # fully invalid
        return th

    def pack_x(xTb, h):
        """[1024, 2048] -> [512, 4096] permuted-block group-major layout."""
        t = xTb.reshape(8, 128, 16, 128)               # (c, p, B, n)
        pm = perm(h)
        blocks = np.array([[4 * g + pm[s] for s in range(4)] for g in range(4)])
        # arr[g, p, c, s, n] = t[c, p, blocks[g, s], n]
        sel = t[:, :, blocks, :]                       # (c, p, g, s, n)
        return np.ascontiguousarray(
            sel.transpose(2, 1, 0, 3, 4).reshape(4 * 128, 8 * 512))

    thrs = [half_thr(0), half_thr(1)]
    in_maps = []
    for core in range(8):
        b, h = core // 2, core % 2
        xTb = np.ascontiguousarray(x[b].T).astype(f16)  # [1024, 2048]
        in_maps.append({
            "sm16": sm16, "sm32": thrs[h], "wpack": wpack,
            "xg": pack_x(xTb, h),
        })
    return in_maps


def kernel(x, Wq, Wk, Wv, L_grav):
    import concourse.bass_utils as bass_utils

    if "nc" not in _CACHE:
        _CACHE["nc"] = _build_nc()
    nc = _CACHE["nc"]
    in_maps = _host_inputs(x, Wq, Wk, Wv, L_grav)

    trace = bool(os.environ.get("BASS_KERNEL_TRACE"))
    if trace:
        bass_utils.upload_artifacts = lambda tmpdir: f"file://{tmpdir}"
    res = bass_utils.run_bass_kernel_spmd(nc, in_maps, list(range(8)), trace=trace)
    if trace:
        _CACHE["exec_time_ns"] = res.exec_time_ns
        _CACHE["mean_exec_time_ns"] = res.mean_exec_time_ns

    out = np.empty((B, N, D_HEAD), np.float32)
    for core in range(8):
        b, h = core // 2, core % 2
        r = res.results[core]["out"]
        for i in range(8):  # local block i -> global block G
            G = 4 * (i // 2) + h + 2 * (i % 2)
            out[b, G * 128:(G + 1) * 128] = r[i * 128:(i + 1) * 128]
    return out
